# revision 14
# baseline (speedup 1.0000x reference)
import sys
import numpy as np

for p in ("/opt/trn_rl_repo",):
    if p not in sys.path:
        sys.path.insert(0, p)

NC_CAP, DC, ROUT, EPS = 16, 32, 3, 1e-7
B, S, DIN, O = 256, 512, 256, 512     # full problem; O = NC_CAP*DC
NCORES = 8
BPC = B // NCORES                     # 32 batches per core
G = 8                                 # batches per pipelined group
NG = BPC // G
SC, IC, OC = S // 128, DIN // 128, O // 128   # 4, 2, 4 chunks

LAST_RESULTS = None
_NC_CACHE = None


def _kernel_numpy(u_vecs, W):
    u = u_vecs.astype(np.float32)
    w = W[0].astype(np.float32)
    uh = np.einsum('bsi,io->bso', u, w)
    uh = uh.reshape(B, S, NC_CAP, DC).transpose(0, 2, 1, 3)
    b = np.zeros((B, NC_CAP, S), dtype=np.float32)
    out = None
    for i in range(ROUT):
        m = b.max(axis=1, keepdims=True)
        e = np.exp(b - m)
        c = e / e.sum(axis=1, keepdims=True)
        o = np.einsum('bni,bnid->bnd', c, uh)
        out = o / np.sqrt((o * o).sum(-1, keepdims=True) + EPS)
        if i < ROUT - 1:
            b = np.einsum('bnd,bnid->bni', out, uh)
    return out.astype(np.float32)


def _build_bass(reps=1):
    import concourse.bass as bass
    import concourse.tile as tile
    from concourse import mybir, bacc
    from contextlib import ExitStack

    f32, bf16 = mybir.dt.float32, mybir.dt.bfloat16
    AF = mybir.ActivationFunctionType
    ALU = mybir.AluOpType
    AX = mybir.AxisListType

    nc = bacc.Bacc()
    u_d = nc.declare_dram_parameter("u", [BPC, S, DIN], f32, isOutput=False)
    w_d = nc.declare_dram_parameter("W", [1, DIN, O], f32, isOutput=False)
    out_d = nc.declare_dram_parameter("out", [BPC, NC_CAP, DC], f32, isOutput=True)

    with ExitStack() as ctx:
        tc = ctx.enter_context(tile.TileContext(nc))
        const = ctx.enter_context(tc.tile_pool(name="const", bufs=1))
        sb_uf = ctx.enter_context(tc.tile_pool(name="sb_uf", bufs=6))
        sb_un = ctx.enter_context(tc.tile_pool(name="sb_un", bufs=2))
        sb_ut = ctx.enter_context(tc.tile_pool(name="sb_ut", bufs=2))
        sb_m = ctx.enter_context(tc.tile_pool(name="sb_m", bufs=2))
        sb_a = ctx.enter_context(tc.tile_pool(name="sb_a", bufs=3))
        sb_b = ctx.enter_context(tc.tile_pool(name="sb_b", bufs=3))
        # PSUM: 8 banks of 2KB/partition, whole-bank allocation per tag*buf
        ps_ut = ctx.enter_context(tc.tile_pool(name="ps_ut", bufs=2, space="PSUM"))
        ps_mq = ctx.enter_context(tc.tile_pool(name="ps_mq", bufs=1, space="PSUM"))
        ps_pr = ctx.enter_context(tc.tile_pool(name="ps_pr", bufs=1, space="PSUM"))
        ps_bt = ctx.enter_context(tc.tile_pool(name="ps_bt", bufs=1, space="PSUM"))
        ps_nr = ctx.enter_context(tc.tile_pool(name="ps_nr", bufs=1, space="PSUM"))
        ps_sb = ctx.enter_context(tc.tile_pool(name="ps_sb", bufs=1, space="PSUM"))
        ps_tr = ctx.enter_context(tc.tile_pool(name="ps_tr", bufs=1, space="PSUM"))

        # ---------------- constants ----------------
        ones = const.tile([128, 512], bf16, tag="ones")
        nc.gpsimd.memset(ones[:], 1.0)
        ident = const.tile([128, 128], bf16, tag="ident")
        nc.gpsimd.affine_select(ident[:], ones[:, 0:128], pattern=[[-1, 128]],
                                compare_op=ALU.is_equal, fill=0.0,
                                base=0, channel_multiplier=1)
        ident_f = const.tile([128, 128], f32, tag="identf")
        nc.vector.tensor_copy(ident_f[:], ident[:])

        # msel[c][p, n'] = 1 iff capsule(128c+p) == n'   (capsule = nd >> 5)
        msel = []
        for c in range(OC):
            t0 = const.tile([128, 16], bf16, tag=f"mselt{c}")
            nc.gpsimd.affine_select(t0[:], ones[:, 0:16], pattern=[[-32, 16]],
                                    compare_op=ALU.is_ge, fill=0.0,
                                    base=128 * c, channel_multiplier=1)
            t1 = const.tile([128, 16], bf16, tag=f"msel{c}")
            nc.gpsimd.affine_select(t1[:], t0[:], pattern=[[32, 16]],
                                    compare_op=ALU.is_ge, fill=0.0,
                                    base=31 - 128 * c, channel_multiplier=-1)
            msel.append(t1)

        # BmaskT[n', nd] = 1 iff capsule(nd) == n'   ([16, 512])
        bmT = const.tile([16, 512], bf16, tag="bmT")
        for c in range(OC):
            tt = const.tile([16, 128], bf16, tag=f"bmTt{c}")
            nc.gpsimd.affine_select(tt[:], ones[0:16, 0:128], pattern=[[1, 128]],
                                    compare_op=ALU.is_ge, fill=0.0,
                                    base=128 * c, channel_multiplier=-32)
            nc.gpsimd.affine_select(bmT[:, 128 * c:128 * (c + 1)], tt[:],
                                    pattern=[[-1, 128]],
                                    compare_op=ALU.is_ge, fill=0.0,
                                    base=31 - 128 * c, channel_multiplier=32)

        eps_t = const.tile([16, 1], f32, tag="eps")
        nc.gpsimd.memset(eps_t[:], EPS)

        # ---------------- W load, cast, transpose ----------------
        wf = const.tile([128, IC, O], f32, tag="wf")
        nc.sync.dma_start(wf[:], w_d[0].rearrange("(c p) o -> p c o", p=128))
        wn = const.tile([128, IC, O], bf16, tag="wn")        # W  [i-part, ic, nd]
        nc.scalar.copy(wn[:], wf[:])
        wT = const.tile([128, OC, IC, 128], bf16, tag="wT")  # W^T [nd-part, c, ic, i]
        for c in range(OC):
            pw = ps_ut.tile([128, IC, 512], bf16, tag="utp")
            for ic in range(IC):
                nc.tensor.transpose(pw[:, ic, 0:128],
                                    wn[:, ic, 128 * c:128 * (c + 1)], ident[:])
                nc.scalar.copy(wT[:, c, ic, :], pw[:, ic, 0:128])

        for gi in range(NG * reps):
            g0 = (gi % NG) * G
            # -------- phase 0: load / cast bf16 / build u^T --------
            un = sb_un.tile([128, SC, G, DIN], bf16, tag="un")   # [s-part, sc, g, i]
            ut = [sb_ut.tile([128, G, S], bf16, tag=f"ut{ic}", name=f"ut{ic}") for ic in range(IC)]
            m1f = [sb_m.tile([128, G], f32, tag=f"m1f{ic}", name=f"m1f{ic}") for ic in range(IC)]
            for g in range(G):
                uf = sb_uf.tile([128, SC, DIN], f32, tag="uf")
                nc.sync.dma_start(uf[:], u_d[g0 + g].rearrange("(sc p) i -> p sc i", p=128))
                nc.gpsimd.tensor_copy(un[:, :, g, :], uf[:])
                pt = ps_ut.tile([128, IC, S], bf16, tag="utp")
                for ic in range(IC):
                    for sc in range(SC):
                        nc.tensor.transpose(pt[:, ic, 128 * sc:128 * (sc + 1)],
                                            un[:, sc, g, 128 * ic:128 * (ic + 1)],
                                            ident[:])
                for ic in range(IC):
                    # copy-cast PSUM->SBUF; row-sum = sum_s u[g,s,i] (for iter 1)
                    nc.scalar.activation(ut[ic][:, g, :], pt[:, ic, :], AF.Copy,
                                         accum_out=m1f[ic][:, g:g + 1])
            m1 = [sb_m.tile([128, G], bf16, tag=f"m1{ic}", name=f"m1{ic}") for ic in range(IC)]
            for ic in range(IC):
                nc.scalar.mul(m1[ic][:], m1f[ic][:], 1.0 / NC_CAP)

            cT = None    # [sc] -> [128, G, 16] bf16
            for rt in range(ROUT):
                # ---- m^T [i-part, g, n] ----
                if rt == 0:
                    nfree = 1
                    mt = m1                      # uniform over n
                else:
                    nfree = 16
                    mq = ps_mq.tile([128, 2 * IC, G, 16], f32, tag="mq")
                    for g in range(G):
                        for ic in range(IC):
                            for sc in range(SC):
                                nc.tensor.matmul(
                                    mq[:, ic, g, :],
                                    un[:, sc, g, 128 * ic:128 * (ic + 1)],
                                    cT[sc][:, g, :],
                                    start=(sc == 0), stop=(sc == SC - 1))
                    mt = [sb_m.tile([128, G, 16], bf16, tag=f"mt{ic}", name=f"mt{ic}") for ic in range(IC)]
                    for ic in range(IC):
                        nc.scalar.copy(mt[ic].rearrange("p a b -> p (a b)"),
                                       mq[:, ic].rearrange("p a b -> p (a b)"))

                # ---- o (pre-squash): W-projection of m, diagonal block ----
                pr = ps_pr.tile([128, OC, 128], f32, tag="pr")
                for c in range(OC):
                    for ic in range(IC):
                        nc.tensor.matmul(
                            pr[:, c, 0:G * nfree],
                            wn[:, ic, 128 * c:128 * (c + 1)],
                            mt[ic].rearrange("p a b -> p (a b)") if nfree > 1 else mt[ic][:],
                            start=(ic == 0), stop=(ic == IC - 1))
                if rt == 0:
                    o_f = [pr[:, c, 0:G] for c in range(OC)]
                else:
                    o_f = []
                    for c in range(OC):
                        tm = sb_a.tile([128, G, 16], bf16, tag=f"xt{c}")
                        nc.vector.tensor_tensor(
                            tm[:],
                            pr[:, c].rearrange("p (g n) -> p g n", g=G),
                            msel[c].unsqueeze(1).broadcast_to((128, G, 16)),
                            op=ALU.mult)
                        of = sb_a.tile([128, G], f32, tag=f"of{c}")
                        nc.vector.reduce_sum(of[:], tm[:], axis=AX.X)
                        o_f.append(of[:])

                # ---- squash scale: 1/sqrt(sum_d o^2 + eps) ----
                sq = []
                for c in range(OC):
                    s_ = sb_a.tile([128, G], bf16, tag=f"sq{c}")
                    nc.scalar.square(s_[:], o_f[c])
                    sq.append(s_)
                nrm = ps_nr.tile([16, G], f32, tag="nrm")
                for c in range(OC):
                    nc.tensor.matmul(nrm[:], msel[c][:], sq[c][:],
                                     start=(c == 0), stop=(c == OC - 1))
                sn = sb_a.tile([16, G], f32, tag="sn")
                nc.scalar.activation(sn[:], nrm[:], AF.Sqrt, bias=eps_t[:])
                rsn = sb_a.tile([16, G], f32, tag="rsn")
                nc.vector.reciprocal(rsn[:], sn[:])
                rsb = sb_a.tile([16, G], bf16, tag="rsb")
                nc.vector.tensor_copy(rsb[:], rsn[:])
                sbc_ps = ps_sb.tile([128, OC, G], f32, tag="sbc")
                for c in range(OC):
                    nc.tensor.matmul(sbc_ps[:, c, :], bmT[:, 128 * c:128 * (c + 1)],
                                     rsb[:], start=True, stop=True)
                sbc = sb_a.tile([128, OC, G], f32, tag="sbcs")
                nc.scalar.copy(sbc.rearrange("p a b -> p (a b)"),
                               sbc_ps.rearrange("p a b -> p (a b)"))

                if rt == ROUT - 1:
                    # ---- final output: out = o * scale, transpose, DMA ----
                    ptr = ps_tr.tile([G, OC, 128], f32, tag="ptr")
                    for c in range(OC):
                        t_ = sb_a.tile([128, G], f32, tag=f"ocf{c}")
                        nc.vector.tensor_tensor(t_[:], o_f[c], sbc[:, c, :], op=ALU.mult)
                        nc.tensor.transpose(ptr[:, c, :], t_[:], ident_f[:])
                    fout = sb_a.tile([G, OC * 128], f32, tag="fout")
                    nc.scalar.copy(fout[:], ptr.rearrange("g c p -> g (c p)"))
                    nc.sync.dma_start(
                        out_d[g0:g0 + G].rearrange("g n d -> g (n d)"), fout[:])
                    continue

                # ---- E = outc (x) msel ; q^T = W^T E ----
                E = []
                for c in range(OC):
                    t_ = sb_a.tile([128, G], bf16, tag=f"ocb{c}")
                    nc.vector.tensor_tensor(t_[:], o_f[c], sbc[:, c, :], op=ALU.mult)
                    e_ = sb_b.tile([128, G, 16], bf16, tag=f"E{c}")
                    nc.vector.tensor_tensor(
                        e_[:],
                        t_.unsqueeze(2).broadcast_to((128, G, 16)),
                        msel[c].unsqueeze(1).broadcast_to((128, G, 16)),
                        op=ALU.mult)
                    E.append(e_)
                qps = ps_mq.tile([128, 2 * IC, G, 16], f32, tag="mq")
                for ic in range(IC):
                    for c in range(OC):
                        nc.tensor.matmul(
                            qps[:, IC + ic].rearrange("p a b -> p (a b)"),
                            wT[:, c, ic, :],
                            E[c].rearrange("p a b -> p (a b)"),
                            start=(c == 0), stop=(c == OC - 1))
                qb = [sb_b.tile([128, G, 16], bf16, tag=f"qb{ic}", name=f"qb{ic}") for ic in range(IC)]
                for ic in range(IC):
                    nc.scalar.copy(qb[ic].rearrange("p a b -> p (a b)"),
                                   qps[:, IC + ic].rearrange("p a b -> p (a b)"))

                # ---- b^T = u q^T  ([s-part, sc, g, n]) ----
                bt = ps_bt.tile([128, SC, G, 16], f32, tag="bt")
                for g in range(G):
                    for sc in range(SC):
                        for ic in range(IC):
                            nc.tensor.matmul(bt[:, sc, g, :],
                                             ut[ic][:, g, 128 * sc:128 * (sc + 1)],
                                             qb[ic][:, g, :],
                                             start=(ic == 0), stop=(ic == IC - 1))

                # ---- softmax over n (groups of 16 in free dim) ----
                ncT = []
                for sc in range(SC):
                    e_ = sb_b.tile([128, G, 16], bf16, tag=f"e{sc}")
                    nc.scalar.activation(e_.rearrange("p a b -> p (a b)"),
                                         bt[:, sc].rearrange("p a b -> p (a b)"), AF.Exp)
                    z_ = sb_b.tile([128, G], f32, tag=f"z{sc}")
                    nc.vector.reduce_sum(z_[:], e_[:], axis=AX.X)
                    rz = sb_b.tile([128, G], f32, tag=f"rz{sc}")
                    nc.vector.reciprocal(rz[:], z_[:])
                    ct = sb_b.tile([128, G, 16], bf16, tag=f"cT{sc}")
                    nc.vector.tensor_tensor(
                        ct[:], e_[:],
                        rz.unsqueeze(2).broadcast_to((128, G, 16)),
                        op=ALU.mult)
                    ncT.append(ct)
                cT = ncT
    nc.finalize()
    return nc


def kernel(u_vecs, W):
    global LAST_RESULTS, _NC_CACHE
    try:
        from concourse.bass_utils import run_bass_kernel_spmd
        if _NC_CACHE is None:
            _NC_CACHE = _build_bass()
        nc = _NC_CACHE
        u = np.ascontiguousarray(u_vecs, dtype=np.float32)
        w = np.ascontiguousarray(W, dtype=np.float32)
        in_maps = [{"u": u[c * BPC:(c + 1) * BPC], "W": w} for c in range(NCORES)]
        res = run_bass_kernel_spmd(nc, in_maps, core_ids=list(range(NCORES)))
        LAST_RESULTS = res
        out = np.concatenate([res.results[c]["out"] for c in range(NCORES)], axis=0)
        return out.astype(np.float32)
    except Exception as ex:
        import traceback
        traceback.print_exc()
        sys.stderr.write(f"[kernel.py] bass path failed ({ex!r}); numpy fallback\n")
        return _kernel_numpy(u_vecs, W)
